# revision 1
# baseline (speedup 1.0000x reference)
"""Self-contained Trainium2 kernel for nn_Block (dense transformer block), 8-way batch-parallel across NeuronCores."""

"""Per-core Bass/Tile program for one transformer block over one batch
element: x[1024, 768] -> out[1024, 768].

Layouts: token-major = tokens on partitions; feature-major = channels on
partitions.  LN runs token-major (free-dim stats via bn_stats), then
PE-transposes into feature-major with the LN gain/bias fused into the
PSUM evacuation on ACT (per-partition scale/bias APs).  q^T,k^T are
produced feature-major (lhsT=w_qkv columns, rhs=h^T); v is produced
token-major (lhsT=h^T) and stored 65-strided with a ones column per head
so the P@v matmul emits softmax denominators in row 64.  Scores S^T are
k-major; exp on ACT over full-q [128,1024] tiles with scale=1/8 and no
max subtraction (scores are O(1) for LN'd inputs).  1/denom via
reciprocal_approx_fast, broadcast across the 64 head rows by a K=1
matmul with ones.  proj runs token-major (lhsT=o^T) with the residual
written in place of x; then LN2 and the same transpose dance.  MLP: u^T
feature-major (lhsT=w_fc1 column slices), gelu(+b_fc1 per-partition) ->
g^T; y^T via lhsT=w_fc2 column slices, rhs=g^T; PE-transpose back +
residual -> out.

All matmul operands are bf16 (measured end-to-end error vs the fp32
reference: ~7e-4 relative).  Weight matrices arrive in DRAM already
cast to bf16 by the host.  Accumulation stays fp32 in PSUM; residuals,
LN stats, softmax denominators, and y^T stay fp32.
"""

import concourse.bass as bass
import concourse.mybir as mybir
from concourse.masks import make_identity

AF = mybir.ActivationFunctionType
ALU = mybir.AluOpType
FP32 = mybir.dt.float32
BF16 = mybir.dt.bfloat16

N, C, H, HD, HID = 1024, 768, 12, 64, 4 * 768
P = 128
TOK = N // P  # 8 token chunks
CT = C // P  # 6 channel chunks
QKT = 2 * C // P  # 12 tiles of [q^T; k^T]
HIDT = HID // P  # 24 hidden chunks
EPS = 1e-5
SCALE = HD ** (-0.5)


def build(nc: bass.Bass, tc, with_b_proj=True, with_b_fc2=True):
    ctx_lp = nc.allow_low_precision(
        reason="bf16 matmul operands, fp32 accum; validated ~7e-4 rel err"
    )
    ctx_lp.__enter__()
    x = nc.dram_tensor("x", [N, C], FP32, kind="ExternalInput").ap()
    ln1_g = nc.dram_tensor("ln1_g", [C], FP32, kind="ExternalInput").ap()
    ln1_b = nc.dram_tensor("ln1_b", [C], FP32, kind="ExternalInput").ap()
    w_qkv = nc.dram_tensor("w_qkv", [C, 3 * C], BF16, kind="ExternalInput").ap()
    w_proj = nc.dram_tensor("w_proj", [C, C], BF16, kind="ExternalInput").ap()
    b_proj = nc.dram_tensor("b_proj", [C], FP32, kind="ExternalInput").ap()
    ln2_g = nc.dram_tensor("ln2_g", [C], FP32, kind="ExternalInput").ap()
    ln2_b = nc.dram_tensor("ln2_b", [C], FP32, kind="ExternalInput").ap()
    w_fc1 = nc.dram_tensor("w_fc1", [C, HID], BF16, kind="ExternalInput").ap()
    b_fc1 = nc.dram_tensor("b_fc1", [HID], FP32, kind="ExternalInput").ap()
    w_fc2 = nc.dram_tensor("w_fc2", [HID, C], BF16, kind="ExternalInput").ap()
    b_fc2 = nc.dram_tensor("b_fc2", [C], FP32, kind="ExternalInput").ap()
    out = nc.dram_tensor("out", [N, C], FP32, kind="ExternalOutput").ap()

    with (
        tc.tile_pool(name="singles", bufs=1) as singles,
        tc.tile_pool(name="xpool", bufs=1) as xpool,
        tc.tile_pool(name="temps", bufs=3) as temps,
        tc.tile_pool(name="stats", bufs=4) as stats,
    ):
        # --- constants -------------------------------------------------
        identB = singles.tile([P, P], BF16, tag="identB", name="identB")
        make_identity(nc, identB)
        identF = singles.tile([P, P], FP32, tag="identF", name="identF")
        make_identity(nc, identF)
        eps_t = singles.tile([P, 1], FP32, tag="eps", name="eps")
        nc.vector.memset(eps_t, EPS)

        def col_load(vec_ap, n_ch, tag):
            """[n_ch*128] DRAM vector -> [128, n_ch] SBUF per-partition."""
            t = singles.tile([P, n_ch], FP32, tag=tag, name=tag)
            nc.sync.dma_start(out=t, in_=vec_ap.rearrange("(c p) -> p c", p=P))
            return t

        def bcast_load(vec_ap, tag):
            """[768] DRAM vector -> [128, 768] broadcast across partitions."""
            t = singles.tile([P, C], FP32, tag=tag, name=tag)
            src = bass.AP(
                tensor=vec_ap.tensor,
                offset=vec_ap.offset,
                ap=[[0, P], *vec_ap.ap],
            )
            nc.sync.dma_start(out=t, in_=src)
            return t

        g1c = col_load(ln1_g, CT, "g1c")
        b1c = col_load(ln1_b, CT, "b1c")
        g2c = col_load(ln2_g, CT, "g2c")
        b2c = col_load(ln2_b, CT, "b2c")
        bf1c = col_load(b_fc1, HIDT, "bf1c")
        bp_b = bcast_load(b_proj, "bp_b") if with_b_proj else None
        bf2_b = bcast_load(b_fc2, "bf2_b") if with_b_fc2 else None
        ones1 = singles.tile([1, P], BF16, tag="ones1", name="ones1")
        nc.vector.memset(ones1, 1.0)

        # --- load x ----------------------------------------------------
        xt = [
            xpool.tile([P, C], FP32, tag=f"x{m}", name=f"x{m}")
            for m in range(TOK)
        ]
        for m in range(TOK):
            nc.sync.dma_start(out=xt[m], in_=x[m * P : (m + 1) * P, :])

        def ln_normalize(src_tile):
            """token-major [128, 768] -> bf16 normalized (x-mu)*rstd."""
            st = stats.tile([P, 3, 6], FP32, tag="bnst", name="bnst")
            src3 = src_tile.rearrange("p (s d) -> p s d", s=3)
            for s in range(3):
                nc.vector.bn_stats(out=st[:, s, :], in_=src3[:, s, :])
            mv = stats.tile([P, 2], FP32, tag="bnmv", name="bnmv")
            nc.vector.bn_aggr(out=mv, in_=st)
            rstd = stats.tile([P, 1], FP32, tag="bnrstd", name="bnrstd")
            nc.scalar.activation(
                out=rstd, in_=mv[:, 1:2], func=AF.Sqrt, bias=eps_t, scale=1.0
            )
            nc.vector.reciprocal(out=rstd, in_=rstd)
            hn = temps.tile([P, C], BF16, tag="hn", name="hn")
            nc.vector.tensor_scalar(
                out=hn, in0=src_tile, scalar1=mv[:, 0:1], scalar2=rstd,
                op0=ALU.subtract, op1=ALU.mult,
            )
            return hn

        def transpose_affine(hn, dstT_tiles, m, gcol, bcol, pspool, tag):
            """transpose bf16 token-major [128,768] into 6 feature-major
            bf16 tiles' column m; g,b applied per-partition on ACT."""
            for c in range(CT):
                tp = pspool.tile([P, P], BF16, tag=tag, name=tag)
                nc.tensor.transpose(tp, hn[:, c * P : (c + 1) * P], identB)
                dst = dstT_tiles[c][:, m * P : (m + 1) * P]
                if c < CT // 2:
                    nc.scalar.activation(
                        out=dst, in_=tp, func=AF.Identity,
                        scale=gcol[:, c : c + 1], bias=bcol[:, c : c + 1],
                    )
                else:
                    nc.vector.tensor_scalar(
                        out=dst, in0=tp, scalar1=gcol[:, c : c + 1],
                        scalar2=bcol[:, c : c + 1], op0=ALU.mult, op1=ALU.add,
                    )

        with tc.tile_pool(name="hTpool", bufs=1) as hTpool:
            # --- LN1 + transpose -> hT; fold b_proj into residual x ----
            hT = [
                hTpool.tile([P, N], BF16, tag=f"hT{c}", name=f"hT{c}")
                for c in range(CT)
            ]
            with tc.tile_pool(name="psA", bufs=4, space="PSUM") as psA:
                for m in range(TOK):
                    hn = ln_normalize(xt[m])
                    transpose_affine(hn, hT, m, g1c, b1c, psA, "trA")
                    if with_b_proj:
                        nc.vector.tensor_tensor(
                            out=xt[m], in0=xt[m], in1=bp_b, op=ALU.add
                        )

            with tc.tile_pool(name="qkTpool", bufs=1) as qkTpool:
                qkT = [
                    qkTpool.tile([P, N], BF16, tag=f"qkT{i}", name=f"qkT{i}")
                    for i in range(QKT)
                ]
                with tc.tile_pool(name="vxpool", bufs=1) as vxpool:
                    # --- qkv: v token-major, ones column per head ------
                    vx = [
                        vxpool.tile(
                            [P, H, HD + 1], BF16, tag=f"vx{m}", name=f"vx{m}"
                        )
                        for m in range(TOK)
                    ]
                    with (
                        tc.tile_pool(name="psV", bufs=2, space="PSUM") as psV,
                        tc.tile_pool(name="wvp", bufs=6) as wvp,
                    ):
                        wv = [
                            wvp.tile([P, C], BF16, tag="wv", name="wv")
                            for k in range(CT)
                        ]
                        for k in range(CT):
                            nc.sync.dma_start(
                                out=wv[k],
                                in_=w_qkv[k * P : (k + 1) * P, 2 * C : 3 * C],
                            )
                        for m in range(TOK):
                            ps = psV.tile([P, C], FP32, tag="vps", name="vps")
                            for k in range(CT):
                                for n0, n1 in ((0, 512), (512, 768)):
                                    nc.tensor.matmul(
                                        ps[:, n0:n1],
                                        lhsT=hT[k][:, m * P : (m + 1) * P],
                                        rhs=wv[k][:, n0:n1],
                                        start=(k == 0),
                                        stop=(k == CT - 1),
                                    )
                            nc.vector.memset(vx[m][:, :, HD : HD + 1], 1.0)
                            nc.vector.tensor_copy(
                                vx[m][:, :, 0:HD],
                                ps.rearrange("p (h d) -> p h d", h=H),
                            )

                    with tc.tile_pool(name="oTpool", bufs=1) as oTpool:
                        # --- interleaved q^T,k^T production + attention -
                        oT = [
                            oTpool.tile(
                                [P, N], BF16, tag=f"oT{c}", name=f"oT{c}"
                            )
                            for c in range(CT)
                        ]
                        with tc.tile_pool(name="wpp", bufs=6) as wpp:
                          wp = [
                              wpp.tile([P, C], BF16, tag="wp", name="wp")
                              for k in range(CT)
                          ]
                          for k in range(CT):
                              nc.sync.dma_start(
                                  out=wp[k],
                                  in_=w_proj[k * P : (k + 1) * P, :],
                              )
                          with (
                            tc.tile_pool(name="psSm", bufs=4, space="PSUM") as psSm,
                            tc.tile_pool(name="psO", bufs=2, space="PSUM") as psO,
                            tc.tile_pool(name="expp", bufs=1) as expp,
                            tc.tile_pool(name="attn_t", bufs=8) as attn_t,
                            tc.tile_pool(name="rsd", bufs=8, space="DRAM") as rsd,
                            tc.tile_pool(name="wqk", bufs=6) as wqkp,
                          ):
                            wq = [
                                wqkp.tile([P, 2 * C], BF16, tag="wqk", name="wqk")
                                for k in range(CT)
                            ]
                            for k in range(CT):
                                nc.sync.dma_start(
                                    out=wq[k],
                                    in_=w_qkv[k * P : (k + 1) * P, 0 : 2 * C],
                                )

                            def make_qkT(i):
                                for h in range(2):
                                    ps = psSm.tile(
                                        [P, 512], FP32, tag="spsm", name="spsm"
                                    )
                                    for k in range(CT):
                                        nc.tensor.matmul(
                                            ps,
                                            lhsT=wq[k][:, i * P : (i + 1) * P],
                                            rhs=hT[k][:, h * 512 : (h + 1) * 512],
                                            start=(k == 0),
                                            stop=(k == CT - 1),
                                        )
                                    nc.vector.tensor_copy(
                                        qkT[i][:, h * 512 : (h + 1) * 512], ps
                                    )

                            for hp in range(H // 2):
                                make_qkT(hp)
                                make_qkT(CT + hp)
                                qt_pair = qkT[hp]
                                kt_pair = qkT[CT + hp]
                                # scores+exp for both heads, row-groups
                                # interleaved so the two 64-partition MMs
                                # overlap in the PE array (full stream BW)
                                expS = {
                                    (sub, kc): expp.tile(
                                        [P, N], BF16,
                                        tag=f"expS{sub}_{kc}",
                                        name=f"expS{sub}_{kc}",
                                    )
                                    for sub in range(2)
                                    for kc in range(TOK)
                                }
                                for kc in range(TOK):
                                    for qh in range(2):
                                        sp2 = [
                                            psSm.tile(
                                                [P, 512], FP32, tag="spsm",
                                                name="spsm",
                                            )
                                            for _ in range(2)
                                        ]
                                        for sub in range(2):
                                            rows = slice(
                                                sub * HD, (sub + 1) * HD
                                            )
                                            nc.tensor.matmul(
                                                sp2[sub],
                                                lhsT=kt_pair[
                                                    rows, kc * P : (kc + 1) * P
                                                ],
                                                rhs=qt_pair[
                                                    rows,
                                                    qh * 512 : (qh + 1) * 512,
                                                ],
                                                start=True,
                                                stop=True,
                                            )
                                        for sub in range(2):
                                            nc.scalar.activation(
                                                out=expS[sub, kc][
                                                    :, qh * 512 : (qh + 1) * 512
                                                ],
                                                in_=sp2[sub],
                                                func=AF.Exp, scale=SCALE,
                                            )
                                for sub in range(2):
                                    head = 2 * hp + sub
                                    ops = psO.tile(
                                        [P, N], FP32, tag="ops", name="ops"
                                    )
                                    for kc in range(TOK):
                                        for qh in range(2):
                                            nc.tensor.matmul(
                                                ops[
                                                    0 : HD + 1,
                                                    qh * 512 : (qh + 1) * 512,
                                                ],
                                                lhsT=vx[kc][:, head, :],
                                                rhs=expS[sub, kc][
                                                    :, qh * 512 : (qh + 1) * 512
                                                ],
                                                start=(kc == 0),
                                                stop=(kc == TOK - 1),
                                            )
                                    for qh in range(2):
                                        qs = slice(qh * 512, (qh + 1) * 512)
                                        lns = attn_t.tile(
                                            [1, 512], FP32, tag="lns",
                                            name="lns",
                                        )
                                        nc.scalar.activation(
                                            out=lns, in_=ops[HD : HD + 1, qs],
                                            func=AF.Ln,
                                        )
                                        rsf = attn_t.tile(
                                            [1, 512], FP32, tag="rsf",
                                            name="rsf",
                                        )
                                        nc.scalar.activation(
                                            out=rsf, in_=lns, func=AF.Exp,
                                            scale=-1.0,
                                        )
                                        rd = rsd.tile(
                                            [1, 512], FP32, tag="rd", name="rd"
                                        )
                                        nc.sync.dma_start(out=rd, in_=rsf)
                                        rbs = attn_t.tile(
                                            [HD, 512], FP32, tag="rbs",
                                            name="rbs",
                                        )
                                        bsrc = bass.AP(
                                            tensor=rd.tensor,
                                            offset=rd.offset,
                                            ap=[[0, HD], *rd.ap[1:]],
                                        )
                                        nc.sync.dma_start(out=rbs, in_=bsrc)
                                        dst = oT[head // 2][
                                            (head % 2) * HD : (head % 2 + 1)
                                            * HD,
                                            qs,
                                        ]
                                        nc.vector.tensor_tensor(
                                            out=dst, in0=ops[0:HD, qs],
                                            in1=rbs, op=ALU.mult,
                                        )

                        # --- proj + residual -> x1 (in place of x) -----
                          with (
                              tc.tile_pool(name="psP", bufs=2, space="PSUM") as psP,
                          ):
                              for m in range(TOK):
                                  ps = psP.tile(
                                      [P, C], FP32, tag="pps", name="pps"
                                  )
                                  for k in range(CT):
                                      for n0, n1 in ((0, 512), (512, 768)):
                                          nc.tensor.matmul(
                                              ps[:, n0:n1],
                                              lhsT=oT[k][:, m * P : (m + 1) * P],
                                              rhs=wp[k][:, n0:n1],
                                              start=(k == 0),
                                              stop=(k == CT - 1),
                                          )
                                  nc.vector.tensor_tensor(
                                      out=xt[m], in0=ps, in1=xt[m], op=ALU.add
                                  )

        x1t = xt  # x tiles now hold x1 = x (+ b_proj) + attn_out

        # --- LN2 -> h2T; fc1+gelu -> gT; fc2 -> y^T -> out -------------
        with tc.tile_pool(name="gTpool", bufs=1) as gTpool:
            gT = [
                gTpool.tile([P, N], BF16, tag=f"gT{i}", name=f"gT{i}")
                for i in range(HIDT)
            ]
            with tc.tile_pool(name="h2Tpool", bufs=1) as h2Tpool:
                h2T = [
                    h2Tpool.tile([P, N], BF16, tag=f"h2T{c}", name=f"h2T{c}")
                    for c in range(CT)
                ]
                with tc.tile_pool(name="psT2", bufs=3, space="PSUM") as psT2:
                    for m in range(TOK):
                        hn = ln_normalize(x1t[m])
                        transpose_affine(hn, h2T, m, g2c, b2c, psT2, "trB")
                        if with_b_fc2:
                            nc.vector.tensor_tensor(
                                out=x1t[m], in0=x1t[m], in1=bf2_b, op=ALU.add
                            )

                # --- fc1 + gelu -> gT ----------------------------------
                with (
                    tc.tile_pool(name="psU", bufs=2, space="PSUM") as psU,
                    tc.tile_pool(name="wf1p", bufs=6) as wf1p,
                ):
                    wf1t = [
                        wf1p.tile([P, HID], BF16, tag="wf1", name="wf1")
                        for k in range(CT)
                    ]
                    for k in range(CT):
                        nc.sync.dma_start(
                            out=wf1t[k], in_=w_fc1[k * P : (k + 1) * P, :]
                        )
                    for mh in range(HIDT):
                        ps = psU.tile([P, N], FP32, tag="ups", name="ups")
                        for k in range(CT):
                            for h in range(2):
                                nc.tensor.matmul(
                                    ps[:, h * 512 : (h + 1) * 512],
                                    lhsT=wf1t[k][:, mh * P : (mh + 1) * P],
                                    rhs=h2T[k][:, h * 512 : (h + 1) * 512],
                                    start=(k == 0),
                                    stop=(k == CT - 1),
                                )
                        nc.scalar.activation(
                            out=gT[mh], in_=ps, func=AF.Gelu,
                            bias=bf1c[:, mh : mh + 1], scale=1.0,
                        )

            # --- fc2 -> y^T; transpose back + residual -> out ----------
            with (
                tc.tile_pool(name="psY", bufs=1, space="PSUM") as psY,
                tc.tile_pool(name="psT3", bufs=2, space="PSUM") as psT3,
                tc.tile_pool(name="yTs", bufs=2) as yTs,
                tc.tile_pool(name="wf2p", bufs=1) as wf2p,
                tc.tile_pool(name="outp", bufs=1) as outp,
            ):
                wf2t = [
                    wf2p.tile([P, C], BF16, tag=f"wf2_{kc}", name=f"wf2_{kc}")
                    for kc in range(HIDT)
                ]
                for kc in range(HIDT):
                    nc.sync.dma_start(
                        out=wf2t[kc], in_=w_fc2[kc * P : (kc + 1) * P, :]
                    )
                outt = [
                    outp.tile([P, C], FP32, tag=f"out{m}", name=f"out{m}")
                    for m in range(TOK)
                ]
                for pp in range(2):  # output-channel halves of 384
                    yps = [
                        psY.tile(
                            [P, N], FP32, tag=f"yps{m3}", name=f"yps{m3}"
                        )
                        for m3 in range(3)
                    ]
                    for kc in range(HIDT):
                        for m3 in range(3):
                            c0 = pp * 384 + m3 * P
                            for h in range(2):
                                nc.tensor.matmul(
                                    yps[m3][:, h * 512 : (h + 1) * 512],
                                    lhsT=wf2t[kc][:, c0 : c0 + P],
                                    rhs=gT[kc][:, h * 512 : (h + 1) * 512],
                                    start=(kc == 0),
                                    stop=(kc == HIDT - 1),
                                )
                    for m3 in range(3):
                        c = pp * 3 + m3
                        ysb = yTs.tile([P, N], FP32, tag="ysb", name="ysb")
                        nc.vector.tensor_copy(ysb, yps[m3])
                        for m in range(TOK):
                            tp = psT3.tile(
                                [P, P], FP32, tag="trC", name="trC"
                            )
                            nc.tensor.transpose(
                                tp, ysb[:, m * P : (m + 1) * P], identF
                            )
                            nc.vector.tensor_tensor(
                                out=outt[m][:, c * P : (c + 1) * P],
                                in0=tp,
                                in1=x1t[m][:, c * P : (c + 1) * P],
                                op=ALU.add,
                            )
                for m in range(TOK):
                    nc.sync.dma_start(
                        out=out[m * P : (m + 1) * P, :], in_=outt[m]
                    )

    ctx_lp.__exit__(None, None, None)
    return out



# ---- wait splitting (walrus allows 1 sync wait/instruction) ----

"""Post-pass: this container's walrus rejects >1 sync wait per instruction.

Tile's sem-assignment freely attaches several waits to one instruction.
Peel all but the last wait onto freshly inserted NoOp instructions on the
same engine, placed immediately before the instruction in its block.

Safety: every wait references a strictly earlier vector-clock tick, and
per-engine instruction streams are tick-ordered, so moving a wait from an
instruction to an immediately preceding same-engine NoOp only strengthens
ordering (the engine blocks slightly earlier); it cannot deadlock.
For DMA instructions the wait moves from the descriptor to the issuing
engine, which delays the enqueue until the sem is reached - conservative
but correct for the same reason.
"""



def split_multi_waits(nc, max_waits: int = 1) -> int:
    n_split = 0
    for f in nc.m.functions:
        for bb in f.blocks:
            insts = list(bb.instructions)
            out = []
            for inst in insts:
                si = inst.sync_info
                waits = list(si.on_wait) if si is not None else []
                if len(waits) > max_waits:
                    n_split += 1
                    peel = waits[:-max_waits]
                    si.on_wait = waits[-max_waits:]
                    for i in range(0, len(peel), max_waits):
                        nop = mybir.InstNoOp(
                            name=f"I-waitfix-{n_split}-{i}",
                            engine=inst.engine,
                            ins=[],
                            outs=[],
                            sync_info=mybir.SyncInfo(
                                on_wait=peel[i : i + max_waits], on_update=[]
                            ),
                        )
                        nc.register_instruction(nop)
                        out.append(nop)
                out.append(inst)
            if len(out) != len(insts):
                bb.instructions[:] = out
    return n_split



# ----------------------------------------------------------------------
# SPMD entry point: full inputs in, full outputs out (8-way batch-parallel)
# ----------------------------------------------------------------------
import numpy as _np
import ml_dtypes as _mld

_N_CORES = 8
_BF16_KEYS = ["w_qkv", "w_proj", "w_fc1", "w_fc2"]
_FP32_KEYS = ["ln1_g", "ln1_b", "b_proj", "ln2_g", "ln2_b", "b_fc1", "b_fc2"]
_WEIGHT_KEYS = _BF16_KEYS + _FP32_KEYS


def _prep_weights(inputs):
    w = {}
    for k in _BF16_KEYS:
        w[k] = _np.ascontiguousarray(
            _np.asarray(inputs[k], dtype=_np.float32).astype(_mld.bfloat16)
        )
    for k in _FP32_KEYS:
        w[k] = _np.ascontiguousarray(_np.asarray(inputs[k], dtype=_np.float32))
    return w


def _build_program(weights):
    import concourse.tile as tile

    nc = bass.Bass("TRN2", target_bir_lowering=False, debug=False,
                   num_devices=_N_CORES)
    with tile.TileContext(nc) as tc:
        build(
            nc, tc,
            with_b_proj=bool(_np.any(weights["b_proj"])),
            with_b_fc2=bool(_np.any(weights["b_fc2"])),
        )
    split_multi_waits(nc)
    return nc


def kernel(**inputs):
    from concourse.bass_utils import run_bass_kernel_spmd

    x = _np.ascontiguousarray(_np.asarray(inputs["x"], dtype=_np.float32))
    assert x.shape == (8, N, C), x.shape
    weights = _prep_weights(inputs)
    nc = _build_program(weights)
    in_maps = [{"x": x[b], **weights} for b in range(_N_CORES)]
    res = run_bass_kernel_spmd(nc, in_maps, list(range(_N_CORES)))
    out = _np.stack([res.results[b]["out"] for b in range(_N_CORES)])
    return out.astype(_np.float32)

